# revision 1
# baseline (speedup 1.0000x reference)
"""Causal single-head attention on 8 Trainium2 NeuronCores (Bass/Tile).

Problem: x[4,2048,1024] fp32, Wq/Wk/Wv[1024,1024];
  q,k,v = x@W.T ; S = q@k.T/sqrt(d) ; causal softmax ; out = P@v.

Sharding: core c -> (batch b=c//2, query-half h=c%2): 1024 queries each.
Each core projects Q/K/V only for its own 1024 rows; the K/V halves are
exchanged between the two cores of a batch with a pairwise AllGather
(replica groups [[0,1],[2,3],[4,5],[6,7]]), so no projection work is
duplicated.

SPMD-uniform causal mask (keys in natural order): additive mask tiles are
built on-core from a per-core scalar m1 in {-1e30 (h=0), 0 (h=1)}:
  keys [0:1024):   M_A = (k <= q_local) ? 0 : m1      (h=0: causal triangle,
                                                       h=1: fully visible)
  keys [1024:2048): M_B = (k' <= q_local) ? m1 : -1e30 (h=0: fully masked,
                                                       h=1: causal triangle)
The device program is identical on all cores; only tensor contents differ.

All matmuls run as float32r (fp32 through the PE transpose-mode datapath:
1 cycle/row at free-dim>=256 vs 4 cycles/row for plain fp32; values are
rounded to the fp32r grid, end-to-end rel err ~3e-4). Inner loops keep the
stationary operand fixed across moving chunks to amortize weight loads.
"""

import os
import sys

sys.path.insert(0, "/opt/trn_rl_repo")

from contextlib import ExitStack

import numpy as np

import concourse.bass as bass
from concourse import bacc
import concourse.mybir as mybir
import concourse.tile as tile
from concourse.bass_utils import run_bass_kernel_spmd

F32 = mybir.dt.float32
F32R = mybir.dt.float32r

B, N, D = 4, 2048, 1024
P = 128          # partition block
NQ = N // 2      # local queries / own rows per core (1024)
ND = D // P      # 8 d-blocks
NO = D // P      # 8 o-blocks
NS = N // P      # 16 key-slot blocks
NKC = N // 512   # 4 key chunks of 512
MASK_VAL = -1.0e30
GROUPS = [[0, 1], [2, 3], [4, 5], [6, 7]]

_CACHE = {}


def _build_program(iters=1, phase="full"):
    nc = bacc.Bacc("TRN2", target_bir_lowering=False, debug=False, num_devices=8)
    xT = nc.dram_tensor("xT", [D, NQ], F32R, kind="ExternalInput").ap()
    wqT = nc.dram_tensor("wqT", [D, D], F32R, kind="ExternalInput").ap()
    wkT = nc.dram_tensor("wkT", [D, D], F32R, kind="ExternalInput").ap()
    wvT = nc.dram_tensor("wvT", [D, D], F32R, kind="ExternalInput").ap()
    m1 = nc.dram_tensor("m1", [P, 1], F32, kind="ExternalInput").ap()
    ident_d = nc.dram_tensor("ident", [P, P], F32R, kind="ExternalInput").ap()
    out = nc.dram_tensor("out", [NQ, D], F32, kind="ExternalOutput").ap()

    with tile.TileContext(nc) as tc:
        if iters == 1:
            _attention_kernel(tc, out, xT, wqT, wkT, wvT, m1, ident_d, phase)
        else:
            with tc.For_i(0, iters, 1):
                _attention_kernel(tc, out, xT, wqT, wkT, wvT, m1, ident_d, phase)
    nc.compile()
    return nc


def _attention_kernel(tc, out, xT, wqT, wkT, wvT, m1, ident_d, phase="full"):
    nc = tc.nc

    with ExitStack() as ctx:
        # ---- constants ----
        const_pool = ctx.enter_context(tc.tile_pool(name="const", bufs=1))
        ident = const_pool.tile([P, P], F32, tag="ident")
        nc.sync.dma_start(ident[:].bitcast(F32R), ident_d[:, :])
        m1_sb = const_pool.tile([P, 1], F32, tag="m1")
        nc.sync.dma_start(m1_sb[:], m1[:, :])
        zeros = const_pool.tile([P, NQ], F32, tag="zeros")
        nc.vector.memset(zeros[:], 0.0)
        m1row = const_pool.tile([P, NQ], F32, tag="m1row")
        nc.vector.tensor_scalar_add(m1row[:], zeros[:], m1_sb[:])
        m1reg = nc.gpsimd.alloc_register("m1reg")
        nc.gpsimd.reg_load(m1reg, m1_sb[0:1, 0:1].bitcast(mybir.dt.int32))

        # DRAM bounce buffers for the pairwise K/V all-gathers
        dram_pool = ctx.enter_context(tc.tile_pool(name="dram", bufs=1, space="DRAM"))
        k_own_d = dram_pool.tile([NQ, D], F32R, tag="k_own")   # rows = o-blocks
        v_own_d = dram_pool.tile([NQ, D], F32R, tag="v_own")   # rows = own slots
        k_g = dram_pool.tile([N, D], F32R, tag="k_g")
        v_g = dram_pool.tile([N, D], F32R, tag="v_g")

        # ================= projections (own 1024 rows only) =================
        with ExitStack() as pctx:
            x_pool = pctx.enter_context(tc.tile_pool(name="xh", bufs=1))
            xh = [
                x_pool.tile([P, NQ], F32, tag=f"xh{d}", name=f"xh{d}")
                for d in range(ND)
            ]
            for d in range(ND):
                nc.sync.dma_start(xh[d][:].bitcast(F32R), xT[d * P : (d + 1) * P, :])

            with ExitStack() as kvctx:
                w_pool = kvctx.enter_context(tc.tile_pool(name="wfull", bufs=1))
                stage_pool = kvctx.enter_context(tc.tile_pool(name="stage", bufs=3))
                psum_kv = kvctx.enter_context(
                    tc.tile_pool(name="psum_kv", bufs=8, space="PSUM")
                )

                # --- K projection: K_own.T [o, own-slot] -> k_own_d ---
                wk = [
                    w_pool.tile([P, D], F32, tag=f"w{d}", name=f"wk{d}")
                    for d in range(ND)
                ]
                for d in range(ND):
                    nc.sync.dma_start(
                        wk[d][:].bitcast(F32R), wkT[d * P : (d + 1) * P, :]
                    )
                for ob in range(NO):
                    kps = [
                        psum_kv.tile([P, 512], F32, tag="pskv", name=f"kps{kc}")
                        for kc in range(2)
                    ]
                    for d in range(ND):
                        for kc in range(2):  # share the stationary wk slice
                            nc.tensor.matmul(
                                kps[kc][:],
                                wk[d][:, ob * P : (ob + 1) * P].bitcast(F32R),
                                xh[d][:, kc * 512 : (kc + 1) * 512].bitcast(F32R),
                                start=(d == 0),
                                stop=(d == ND - 1),
                            )
                    kst = stage_pool.tile([P, NQ], F32, tag="stage")
                    for kc in range(2):
                        nc.scalar.copy(
                            kst[:, kc * 512 : (kc + 1) * 512].bitcast(F32R), kps[kc][:]
                        )
                    nc.sync.dma_start(
                        k_own_d[ob * P : (ob + 1) * P, :], kst[:].bitcast(F32R)
                    )

                # --- V projection: V_own [own-slot, o] -> v_own_d ---
                wv = [
                    w_pool.tile([P, D], F32, tag=f"wv{d}", name=f"wv{d}")
                    for d in range(ND)
                ]
                for d in range(ND):
                    nc.sync.dma_start(
                        wv[d][:].bitcast(F32R), wvT[d * P : (d + 1) * P, :]
                    )
                for sblk in range(8):
                    vps = [
                        psum_kv.tile([P, 512], F32, tag="pskv", name=f"vps{oc}")
                        for oc in range(2)
                    ]
                    for d in range(ND):
                        for oc in range(2):  # share the stationary xh slice
                            nc.tensor.matmul(
                                vps[oc][:],
                                xh[d][:, sblk * P : (sblk + 1) * P].bitcast(F32R),
                                wv[d][:, oc * 512 : (oc + 1) * 512].bitcast(F32R),
                                start=(d == 0),
                                stop=(d == ND - 1),
                            )
                    vst = stage_pool.tile([P, NQ], F32, tag="stage")
                    for oc in range(2):
                        nc.scalar.copy(
                            vst[:, oc * 512 : (oc + 1) * 512].bitcast(F32R), vps[oc][:]
                        )
                    nc.sync.dma_start(
                        v_own_d[sblk * P : (sblk + 1) * P, :], vst[:].bitcast(F32R)
                    )

            # --- pairwise all-gathers (start as soon as the spills land) ---
            if phase != "nocoll":
                nc.gpsimd.collective_compute(
                    "AllGather",
                    mybir.AluOpType.bypass,
                    replica_groups=GROUPS,
                    ins=[k_own_d.opt()],
                    outs=[k_g.opt()],
                )
                nc.gpsimd.collective_compute(
                    "AllGather",
                    mybir.AluOpType.bypass,
                    replica_groups=GROUPS,
                    ins=[v_own_d.opt()],
                    outs=[v_g.opt()],
                )
            else:
                # timing-only variant: local copy stands in for the exchange
                nc.sync.dma_start(k_g[0:NQ, :], k_own_d[:, :])
                nc.sync.dma_start(k_g[NQ:N, :], k_own_d[:, :])
                nc.sync.dma_start(v_g[0:NQ, :], v_own_d[:, :])
                nc.sync.dma_start(v_g[NQ:N, :], v_own_d[:, :])

            # --- Q projection (QT persists; streamed wq) ---
            qt_pool = ctx.enter_context(tc.tile_pool(name="qt", bufs=1, side="right"))
            QT = [
                qt_pool.tile([P, NQ], F32, tag=f"qt{ob}", name=f"qt{ob}")
                for ob in range(NO)
            ]
            with ExitStack() as qctx:
                wq_pool = qctx.enter_context(tc.tile_pool(name="wqs", bufs=1))
                psum_q = qctx.enter_context(
                    tc.tile_pool(name="psum_q", bufs=1, space="PSUM")
                )
                wqf = [
                    wq_pool.tile([P, D], F32, tag=f"wqf{d}", name=f"wqf{d}")
                    for d in range(ND)
                ]
                for d in range(ND):
                    nc.sync.dma_start(
                        wqf[d][:].bitcast(F32R), wqT[d * P : (d + 1) * P, :]
                    )
                for qpass in range(2):
                    qp = [
                        psum_q.tile([P, 512], F32, tag=f"psq{i}", name=f"psq{i}")
                        for i in range(8)
                    ]
                    for d in range(ND):
                        for obi in range(4):
                            o0 = qpass * 512 + obi * P
                            for qc in range(2):
                                nc.tensor.matmul(
                                    qp[obi * 2 + qc][:],
                                    wqf[d][:, o0 : o0 + P].bitcast(F32R),
                                    xh[d][:, qc * 512 : (qc + 1) * 512].bitcast(F32R),
                                    start=(d == 0),
                                    stop=(d == ND - 1),
                                )
                    for obi in range(4):
                        ob = qpass * 4 + obi
                        for qc in range(2):
                            nc.scalar.copy(
                                QT[ob][:, qc * 512 : (qc + 1) * 512].bitcast(F32R),
                                qp[obi * 2 + qc][:],
                            )

        # ---- load gathered K/V into resident SBUF tiles ----
        kt_pool = ctx.enter_context(tc.tile_pool(name="kt", bufs=1))
        v_pool = ctx.enter_context(tc.tile_pool(name="v", bufs=1))
        KT = [
            kt_pool.tile([P, N], F32, tag=f"kt{ob}", name=f"kt{ob}")
            for ob in range(NO)
        ]
        V = [
            v_pool.tile([P, D], F32, tag=f"v{sb}", name=f"v{sb}") for sb in range(NS)
        ]
        for ob in range(NO):
            for hh in range(2):
                nc.sync.dma_start(
                    KT[ob][:, hh * NQ : (hh + 1) * NQ].bitcast(F32R),
                    k_g[hh * NQ + ob * P : hh * NQ + (ob + 1) * P, :],
                )
        for sb in range(NS):
            nc.sync.dma_start(V[sb][:].bitcast(F32R), v_g[sb * P : (sb + 1) * P, :])

        # ================= attention =================
        if phase == "proj":
            return
        with ExitStack() as actx:
            s_pool = actx.enter_context(tc.tile_pool(name="s", bufs=2))
            mask_pool = actx.enter_context(tc.tile_pool(name="mask", bufs=2))
            stat_pool = actx.enter_context(tc.tile_pool(name="stat", bufs=4))
            pt_pool = actx.enter_context(tc.tile_pool(name="pt", bufs=2))
            o_pool = actx.enter_context(tc.tile_pool(name="o", bufs=1))
            psum_s = actx.enter_context(tc.tile_pool(name="psum_s", bufs=4, space="PSUM"))
            psum_t = actx.enter_context(tc.tile_pool(name="psum_t", bufs=2, space="PSUM"))
            psum_o = actx.enter_context(tc.tile_pool(name="psum_o", bufs=2, space="PSUM"))

            for qb in range(NQ // P):  # 8 query blocks
                # additive causal masks for this q-block (see module docstring)
                MA = mask_pool.tile([P, NQ], F32, tag="ma")
                nc.gpsimd.affine_select(
                    out=MA[:],
                    in_=zeros[:],
                    compare_op=mybir.AluOpType.is_ge,
                    fill=m1reg,
                    base=qb * P,
                    pattern=[[-1, NQ]],
                    channel_multiplier=1,
                )
                MB = mask_pool.tile([P, NQ], F32, tag="mb")
                nc.gpsimd.affine_select(
                    out=MB[:],
                    in_=m1row[:],
                    compare_op=mybir.AluOpType.is_ge,
                    fill=MASK_VAL,
                    base=qb * P,
                    pattern=[[-1, NQ]],
                    channel_multiplier=1,
                )

                S = s_pool.tile([P, N], F32, tag="s")
                # scores: S[q, k] = sum_o QT[o, q] * KT[o, k]
                sps = [
                    psum_s.tile([P, 512], F32, tag="pss", name=f"sps{kc}")
                    for kc in range(NKC)
                ]
                for ob in range(NO):
                    for kc in range(NKC):  # share the stationary QT slice
                        nc.tensor.matmul(
                            sps[kc][:],
                            QT[ob][:, qb * P : (qb + 1) * P].bitcast(F32R),
                            KT[ob][:, kc * 512 : (kc + 1) * 512].bitcast(F32R),
                            start=(ob == 0),
                            stop=(ob == NO - 1),
                        )
                for kc in range(NKC):
                    M = MA if kc < 2 else MB
                    nc.vector.tensor_tensor(
                        S[:, kc * 512 : (kc + 1) * 512].bitcast(F32R),
                        sps[kc][:],
                        M[:, (kc % 2) * 512 : (kc % 2 + 1) * 512],
                        mybir.AluOpType.add,
                    )
                neg_max = stat_pool.tile([P, 1], F32, tag="negmax")
                nc.vector.reduce_max(
                    neg_max[:], S[:], axis=mybir.AxisListType.X, negate=True
                )
                zrow = stat_pool.tile([P, 1], F32, tag="zrow")
                nc.scalar.activation(
                    S[:].bitcast(F32R),
                    S[:],
                    mybir.ActivationFunctionType.Exp,
                    bias=neg_max[:],
                    scale=1.0,
                    accum_out=zrow[:],
                )
                rz = stat_pool.tile([P, 1], F32, tag="rz")
                nc.vector.reciprocal(rz[:], zrow[:])

                # AV: O[q, o] = sum_k P[q, k] V[k, o]
                op0 = psum_o.tile([P, 512], F32, tag="pso", name="op0")
                op1 = psum_o.tile([P, 512], F32, tag="pso", name="op1")
                for sb in range(NS):
                    tp = psum_t.tile([P, P], F32, tag="pst")
                    nc.tensor.transpose(
                        tp[:].bitcast(F32R),
                        S[:, sb * P : (sb + 1) * P].bitcast(F32R),
                        ident[:].bitcast(F32R),
                    )
                    pt = pt_pool.tile([P, P], F32, tag="pt")
                    nc.vector.tensor_copy(pt[:].bitcast(F32R), tp[:])
                    for oc, op in ((0, op0), (1, op1)):
                        nc.tensor.matmul(
                            op[:],
                            pt[:].bitcast(F32R),
                            V[sb][:, oc * 512 : (oc + 1) * 512].bitcast(F32R),
                            start=(sb == 0),
                            stop=(sb == NS - 1),
                        )
                O = o_pool.tile([P, D], F32, tag="o")
                nc.vector.tensor_scalar_mul(O[:, 0:512], op0[:], rz[:])
                nc.vector.tensor_scalar_mul(O[:, 512:1024], op1[:], rz[:])
                nc.sync.dma_start(out[qb * P : (qb + 1) * P, :], O[:])


def _get_program(iters=1, phase="full"):
    key = ("nc", iters, phase)
    if key not in _CACHE:
        _CACHE[key] = _build_program(iters, phase)
    return _CACHE[key]


def _host_prep(x, Wq, Wk, Wv):
    scale = np.float32(1.0 / np.sqrt(np.float32(D)))
    wqT = np.ascontiguousarray((np.asarray(Wq, np.float32) * scale).T)
    wkT = np.ascontiguousarray(np.asarray(Wk, np.float32).T)
    wvT = np.ascontiguousarray(np.asarray(Wv, np.float32).T)
    ident = np.eye(P, dtype=np.float32)
    in_maps = []
    for c in range(8):
        b, h = c // 2, c % 2
        xo = np.asarray(x[b, h * NQ : (h + 1) * NQ], dtype=np.float32)
        in_maps.append(
            {
                "xT": np.ascontiguousarray(xo.T),
                "wqT": wqT,
                "wkT": wkT,
                "wvT": wvT,
                "m1": np.full((P, 1), MASK_VAL if h == 0 else 0.0, np.float32),
                "ident": ident,
            }
        )
    return in_maps


def kernel(x, Wq, Wk, Wv):
    nc = _get_program()
    in_maps = _host_prep(x, Wq, Wk, Wv)
    res = run_bass_kernel_spmd(nc, in_maps, list(range(8)))
    _CACHE["last_results"] = res
    out = np.empty((B, N, D), np.float32)
    for c in range(8):
        b, h = c // 2, c % 2
        out[b, h * NQ : (h + 1) * NQ] = res.results[c]["out"]
    return out

